# revision 32
# baseline (speedup 1.0000x reference)
"""LocallyConnected2d Trainium2 kernel.

Problem: out[b,o,h,w] = sum_{c,i,j} xpad[b,c,h+i,w+j] * weights[h,w,o,c,i,j] + bias[o,h,w]
  B=32, C=32, O=32, H=W=64, K=3, PAD=1, OH=OW=64.

Sharding: each of the 8 cores owns a band of 8 output rows (OH split), with the
matching 10-row input halo. Weights (the dominant traffic) split 1/8 per core
with zero redundancy, streamed as fp16.

Device compute: per output location (h,w) the contraction over (c,i,j)=288 is
split into 3 matmuls of K=96 (tap col j fixed, contraction over (c,i)),
accumulated in PSUM. M=o=32, N=b=32. The 4 locations of an ow-group share the
PE array via col-group tile_position packing. Bias is applied by one rank-16
matmul per output row: lhsT = bias[16g, 128(w4,o)], rhs = g-indicator
[16, 512(g,b)], writing the initial PSUM contents (start=True). The scalar
(activation) engine downcasts PSUM fp32 -> fp16 SBUF tiles, DMAed out in two
0.5 MiB stores. All layout transforms are host-side numpy; every DMA is a
clean 96/16/128-partition spray (one sem per sync-queue load, no recycling).

Host-prepped per-core layouts:
  xp [96, 8*66*32]  : [(c,i), r', w', b] = xpad[b, c, 8d+r'+i, w']
  wp [96, 8*16*384] : [(c,i), h, g, j, w4, o] = weights[8d+h, 4g+w4, o, c, i, j]
  bp [16, 8*128+512]: [g, (h, w4, o)] = bias[o, 8d+h, 4g+w4] ++ kron(I16, 1_32)
  op [2, 128, 2048] : [s, (w4,o), hh, g, b] fp16 (output, h = 4s+hh)
"""

import sys

if "/opt/trn_rl_repo" not in sys.path:
    sys.path.insert(0, "/opt/trn_rl_repo")

import numpy as np

B = 32
C = 32
O = 32
H = W = 64
KK = 3
NCORES = 8
RP = H // NCORES      # output rows per core
W2 = W + 2            # padded row width
P = 96                # contraction partitions (c,i)
NG = W // 4           # ow groups of 4
XGROUPS = [(0, 1), (1, 3), (4, 4)]          # x load groups (start row, nrows)
NWARM = 0                                   # PE warmup matmuls (DVFS pre-ramp)
BCOLS = RP * 128 + 512                      # bias tensor free dim
CHAIN = 2                                   # load chaining depth (0 = off)
W_FP16 = False
W_PAIRS = True                              # weights dtype: fp16 vs fp8e4

_built = {}


def _build():
    if "nc" in _built:
        return _built["nc"]
    import concourse.tile as tile
    from concourse.tile import add_dep_helper
    from concourse import bacc, mybir

    nc = bacc.Bacc("TRN2", target_bir_lowering=False, debug=False,
                   num_devices=NCORES)
    f16 = mybir.dt.float16
    f32 = mybir.dt.float32
    f8 = mybir.dt.float8e4
    WDT = f16 if W_FP16 else f8
    xp = nc.dram_tensor("xp", [P, RP * W2 * B], f16, kind="ExternalInput")
    wp = nc.dram_tensor("wp", [P, RP * NG * 384], WDT, kind="ExternalInput")
    bp = nc.dram_tensor("bp", [16, BCOLS], f16, kind="ExternalInput")
    op = nc.dram_tensor("op", [128, RP * NG * 32], f16,
                        kind="ExternalOutput")

    HF = NG * 384        # free elems per h row in wp
    with tile.TileContext(nc) as tc:
        with tc.tile_pool(name="xpool", bufs=1) as xpool, \
             tc.tile_pool(name="wpool", bufs=1) as wpool, \
             tc.tile_pool(name="bpool", bufs=1) as bpool, \
             tc.tile_pool(name="opool", bufs=3) as opool, \
             tc.tile_pool(name="ppool", bufs=4, space="PSUM") as ppool:
            xt = xpool.tile([P, RP * W2 * B], f16, tag="x")
            wt = wpool.tile([P, RP * HF], WDT, tag="w")
            bt = bpool.tile([16, BCOLS], f16, tag="b")
            wu = bpool.tile([1, 64], f16, tag="wu")

            # memzero first on the scalar queue: the PE warmup stream
            # depends only on it, so warmups start ~1us into the kernel.
            nc.scalar.memzero(wu[:])

            # loads: sync queue = exactly the 8 weight loads (8 HWDGE sems,
            # no slot recycling); scalar queue = bias + x + copies + stores.
            nc.scalar.dma_start(bt[:], bp.ap())

            loadq = []

            def load_w(h, n=1):
                cols = slice(h * HF, (h + n) * HF)
                loadq.append(nc.sync.dma_start(wt[:, cols], wp.ap()[:, cols]))

            def load_xg(gi):
                r0, nr = XGROUPS[gi]
                cols = slice(r0 * W2 * B, (r0 + nr) * W2 * B)
                loadq.append(nc.scalar.dma_start(xt[:, cols], xp.ap()[:, cols]))

            if W_PAIRS:
                load_w(0, 2)
                load_xg(0)
                load_w(2, 2)
                load_xg(1)
                load_w(4, 2)
                load_xg(2)
                load_w(6, 2)
            else:
                load_w(0)
                load_xg(0)
                load_w(1)
                load_xg(1)
                load_w(2)
                load_xg(2)
                for h in range(3, RP):
                    load_w(h)
            if CHAIN:
                for k in range(CHAIN, len(loadq)):
                    add_dep_helper(loadq[k].ins, loadq[k - CHAIN].ins,
                                   sync=True, reason="load chain")

            # PE DVFS warmup: dummy matmuls on zeroed scratch keep the
            # tensor engine busy while the first loads land, so the real
            # matmul stream starts at full clock.
            warm_last = None
            pw = None
            if NWARM:
                pw = ppool.tile([32, 32], f32, tag="ps")
            for k in range(NWARM):
                warm_last = nc.tensor.matmul(
                    pw[:, :], wu[0:1, 0:32], wu[0:1, 32:64],
                    start=True, stop=True, skip_group_check=True,
                    tile_position=(0, 0),
                )

            ot = None
            for h in range(RP):
                ps = ppool.tile([128, NG * 32], f32, tag="ps")
                bm = nc.tensor.matmul(
                    ps[:, :],
                    bt[:, h * 128:(h + 1) * 128],
                    bt[:, RP * 128:RP * 128 + 512],
                    start=True, stop=False,
                    skip_group_check=True,
                    tile_position=(0, 0),
                )
                if h == 0 and warm_last is not None:
                    add_dep_helper(bm.ins, warm_last.ins, sync=True,
                                   reason="warmup precedes real stream")
                # w4 innermost: consecutive matmuls hit different PE column
                # groups so LDWEIGHTS overlaps the neighboring group's MATMUL
                for g in range(NG):
                    for j in range(KK):
                        for w4 in range(4):
                            wo = h * HF + g * 384 + w4 * 32
                            xo = (h * W2 + 4 * g + w4) * B
                            nc.tensor.matmul(
                                ps[32 * w4:32 * w4 + 32, 32 * g:32 * g + 32],
                                wt[:, wo + j * 128:wo + j * 128 + 32],
                                xt[:, xo + j * B:xo + j * B + 32],
                                start=False,
                                stop=(j == KK - 1),
                                skip_group_check=True,
                                tile_position=(0, 32 * w4),
                            )
                # stores in (4, 2, 2) row groups: short final store
                if h in (0, 4, 6):
                    onr = 4 if h == 0 else 2
                    obase = h * NG * 32
                    ot = opool.tile([128, onr * NG * 32], f16, tag="o")
                off = h * NG * 32 - obase
                nc.scalar.copy(ot[:, off:off + NG * 32], ps[:])
                if h in (3, 5, 7):
                    nc.scalar.dma_start(
                        op.ap()[:, obase:(h + 1) * NG * 32], ot[:])
    nc.compile()
    _built["nc"] = nc
    return nc


def prep_inputs(x, weights, bias):
    """Host-side shard + layout prep. Returns list of 8 in_maps."""
    x = np.asarray(x, dtype=np.float32)
    weights = np.asarray(weights, dtype=np.float32)
    bias = np.asarray(bias, dtype=np.float32)
    xpad = np.zeros((B, C, H + 2, W2), dtype=np.float32)
    xpad[:, :, 1:H + 1, 1:W + 1] = x
    ind = np.kron(np.eye(16, dtype=np.float16), np.ones(32, dtype=np.float16))
    in_maps = []
    for d in range(NCORES):
        blk = xpad[:, :, RP * d:RP * d + RP + 2, :]       # [b, c, 10, 66]
        xprep = np.empty((C, KK, RP, W2, B), dtype=np.float16)
        for i in range(KK):
            xprep[:, i] = blk[:, :, i:i + RP, :].transpose(1, 2, 3, 0)
        xprep = xprep.reshape(P, RP * W2 * B)

        wd = weights[RP * d:RP * d + RP]                  # [8, 64, 32, 32, 3, 3]
        wd = wd.reshape(RP, NG, 4, O, C, KK, KK)          # h, g, w4, o, c, i, j
        wci = wd.transpose(4, 5, 0, 1, 6, 2, 3)           # c, i, h, g, j, w4, o
        import ml_dtypes
        wdt = np.float16 if W_FP16 else ml_dtypes.float8_e4m3
        wprep = np.ascontiguousarray(wci).astype(wdt).reshape(
            P, RP * NG * 384)

        bd = bias[:, RP * d:RP * d + RP, :].reshape(O, RP, NG, 4)
        U = bd.transpose(2, 1, 3, 0).reshape(NG, RP * 128)  # g, (h, w4, o)
        bprep = np.concatenate(
            [U.astype(np.float16), ind], axis=1)            # [16, 8*128+512]
        in_maps.append({"xp": xprep, "wp": wprep, "bp": bprep})
    return in_maps


def assemble_output(results):
    """results: list of 8 dicts with 'op' [128, 8*512] -> full [B,O,H,W]."""
    out = np.empty((B, O, H, W), dtype=np.float32)
    for d in range(NCORES):
        arr = np.asarray(results[d]["op"]).reshape(4, O, RP, NG, B)
        # [w4, o, h, g, b] -> [b, o, h, g, w4]
        out[:, :, RP * d:RP * d + RP, :] = (
            arr.transpose(4, 1, 2, 3, 0).reshape(B, O, RP, W)
            .astype(np.float32))
    return out


def _ensure_ntff_hook():
    """The agent image's antenv lacks axon_hooks; inject it and register the
    ctypes NTFF hook (same recipe as trn_agent_boot.trn_boot)."""
    try:
        from antenv.axon_hooks import get_axon_ntff_profile_hook  # noqa: F401
        return
    except ImportError:
        pass
    import types
    import ctypes
    import contextlib

    mod = types.ModuleType("antenv.axon_hooks")
    mod._hook = None

    def set_axon_ntff_profile_hook(h):
        mod._hook = h

    def get_axon_ntff_profile_hook():
        return mod._hook

    mod.set_axon_ntff_profile_hook = set_axon_ntff_profile_hook
    mod.get_axon_ntff_profile_hook = get_axon_ntff_profile_hook
    sys.modules["antenv.axon_hooks"] = mod
    import antenv

    antenv.axon_hooks = mod

    so_path = "/opt/axon/libaxon_pjrt.so"
    try:
        lib = ctypes.CDLL(so_path)
    except OSError:
        return
    if not hasattr(lib, "axon_start_nrt_profile"):
        return
    lib.axon_start_nrt_profile.argtypes = [
        ctypes.POINTER(ctypes.c_int64), ctypes.c_size_t]
    lib.axon_start_nrt_profile.restype = ctypes.c_int64
    lib.axon_stop_nrt_profile.argtypes = [ctypes.c_char_p]
    lib.axon_stop_nrt_profile.restype = ctypes.c_int64

    @contextlib.contextmanager
    def _hook(output_dir, device_ids):
        import jax

        jax.devices()
        if device_ids:
            ids = (ctypes.c_int64 * len(device_ids))(*device_ids)
            rc = lib.axon_start_nrt_profile(ids, len(device_ids))
        else:
            rc = lib.axon_start_nrt_profile(None, 0)
        if rc != 0:
            raise RuntimeError(f"axon_start_nrt_profile rc={rc}")
        try:
            yield
        finally:
            n = lib.axon_stop_nrt_profile(str(output_dir).encode())
            print(f"ntff profile: {n} file(s) written to {output_dir}")

    mod.set_axon_ntff_profile_hook(_hook)


def run(inputs, trace=False, **kwargs):
    from concourse.bass_utils import run_bass_kernel_spmd

    if trace:
        _ensure_ntff_hook()
    nc = _build()
    in_maps = prep_inputs(inputs["x"], inputs["weights"], inputs["bias"])
    res = run_bass_kernel_spmd(nc, in_maps, list(range(NCORES)),
                               trace=trace, **kwargs)
    return assemble_output(res.results), res


def kernel(**inputs):
    out, _ = run(inputs)
    return out


# revision 33
# speedup vs baseline: 1.0426x; 1.0426x over previous
"""LocallyConnected2d Trainium2 kernel.

Problem: out[b,o,h,w] = sum_{c,i,j} xpad[b,c,h+i,w+j] * weights[h,w,o,c,i,j] + bias[o,h,w]
  B=32, C=32, O=32, H=W=64, K=3, PAD=1, OH=OW=64.

Sharding: each of the 8 cores owns a band of 8 output rows (OH split), with the
matching 10-row input halo. Weights (the dominant traffic) split 1/8 per core
with zero redundancy, streamed as fp16.

Device compute: per output location (h,w) the contraction over (c,i,j)=288 is
split into 3 matmuls of K=96 (tap col j fixed, contraction over (c,i)),
accumulated in PSUM. M=o=32, N=b=32. The 4 locations of an ow-group share the
PE array via col-group tile_position packing. Bias is applied by one rank-16
matmul per output row: lhsT = bias[16g, 128(w4,o)], rhs = g-indicator
[16, 512(g,b)], writing the initial PSUM contents (start=True). The scalar
(activation) engine downcasts PSUM fp32 -> fp16 SBUF tiles, DMAed out in two
0.5 MiB stores. All layout transforms are host-side numpy; every DMA is a
clean 96/16/128-partition spray (one sem per sync-queue load, no recycling).

Host-prepped per-core layouts:
  xp [96, 8*66*32]  : [(c,i), r', w', b] = xpad[b, c, 8d+r'+i, w']
  wp [96, 8*16*384] : [(c,i), h, g, j, w4, o] = weights[8d+h, 4g+w4, o, c, i, j]
  bp [16, 8*128+512]: [g, (h, w4, o)] = bias[o, 8d+h, 4g+w4] ++ kron(I16, 1_32)
  op [2, 128, 2048] : [s, (w4,o), hh, g, b] fp16 (output, h = 4s+hh)
"""

import sys

if "/opt/trn_rl_repo" not in sys.path:
    sys.path.insert(0, "/opt/trn_rl_repo")

import numpy as np

B = 32
C = 32
O = 32
H = W = 64
KK = 3
NCORES = 8
RP = H // NCORES      # output rows per core
W2 = W + 2            # padded row width
P = 96                # contraction partitions (c,i)
NG = W // 4           # ow groups of 4
XGROUPS = [(0, 1), (1, 3), (4, 4)]          # x load groups (start row, nrows)
NWARM = 0                                   # PE warmup matmuls (DVFS pre-ramp)
BCOLS = RP * 128 + 512                      # bias tensor free dim
CHAIN = 2                                   # load chaining depth (0 = off)
W_FP16 = False
W_PAIRS = False                              # weights dtype: fp16 vs fp8e4

_built = {}


def _build():
    if "nc" in _built:
        return _built["nc"]
    import concourse.tile as tile
    from concourse.tile import add_dep_helper
    from concourse import bacc, mybir

    nc = bacc.Bacc("TRN2", target_bir_lowering=False, debug=False,
                   num_devices=NCORES)
    f16 = mybir.dt.float16
    f32 = mybir.dt.float32
    f8 = mybir.dt.float8e4
    WDT = f16 if W_FP16 else f8
    xp = nc.dram_tensor("xp", [P, RP * W2 * B], f16, kind="ExternalInput")
    wp = nc.dram_tensor("wp", [P, RP * NG * 384], WDT, kind="ExternalInput")
    bp = nc.dram_tensor("bp", [16, BCOLS], f16, kind="ExternalInput")
    op = nc.dram_tensor("op", [128, RP * NG * 32], f16,
                        kind="ExternalOutput")

    HF = NG * 384        # free elems per h row in wp
    with tile.TileContext(nc) as tc:
        with tc.tile_pool(name="xpool", bufs=1) as xpool, \
             tc.tile_pool(name="wpool", bufs=1) as wpool, \
             tc.tile_pool(name="bpool", bufs=1) as bpool, \
             tc.tile_pool(name="opool", bufs=3) as opool, \
             tc.tile_pool(name="ppool", bufs=4, space="PSUM") as ppool:
            xt = xpool.tile([P, RP * W2 * B], f16, tag="x")
            wt = wpool.tile([P, RP * HF], WDT, tag="w")
            bt = bpool.tile([16, BCOLS], f16, tag="b")
            wu = bpool.tile([1, 64], f16, tag="wu")

            # memzero first on the scalar queue: the PE warmup stream
            # depends only on it, so warmups start ~1us into the kernel.
            nc.scalar.memzero(wu[:])

            # loads: sync queue = exactly the 8 weight loads (8 HWDGE sems,
            # no slot recycling); scalar queue = bias + x + copies + stores.
            nc.scalar.dma_start(bt[:], bp.ap())

            loadq = []

            def load_w(h, n=1):
                cols = slice(h * HF, (h + n) * HF)
                loadq.append(nc.sync.dma_start(wt[:, cols], wp.ap()[:, cols]))

            def load_xg(gi):
                r0, nr = XGROUPS[gi]
                cols = slice(r0 * W2 * B, (r0 + nr) * W2 * B)
                loadq.append(nc.scalar.dma_start(xt[:, cols], xp.ap()[:, cols]))

            if W_PAIRS:
                load_w(0, 2)
                load_xg(0)
                load_w(2, 2)
                load_xg(1)
                load_w(4, 2)
                load_xg(2)
                load_w(6, 2)
            else:
                load_w(0)
                load_xg(0)
                load_w(1)
                load_xg(1)
                load_w(2)
                load_xg(2)
                for h in range(3, RP):
                    load_w(h)
            if CHAIN:
                for k in range(CHAIN, len(loadq)):
                    add_dep_helper(loadq[k].ins, loadq[k - CHAIN].ins,
                                   sync=True, reason="load chain")

            # PE DVFS warmup: dummy matmuls on zeroed scratch keep the
            # tensor engine busy while the first loads land, so the real
            # matmul stream starts at full clock.
            warm_last = None
            pw = None
            if NWARM:
                pw = ppool.tile([32, 32], f32, tag="ps")
            for k in range(NWARM):
                warm_last = nc.tensor.matmul(
                    pw[:, :], wu[0:1, 0:32], wu[0:1, 32:64],
                    start=True, stop=True, skip_group_check=True,
                    tile_position=(0, 0),
                )

            ot = None
            for h in range(RP):
                ps = ppool.tile([128, NG * 32], f32, tag="ps")
                bm = nc.tensor.matmul(
                    ps[:, :],
                    bt[:, h * 128:(h + 1) * 128],
                    bt[:, RP * 128:RP * 128 + 512],
                    start=True, stop=False,
                    skip_group_check=True,
                    tile_position=(0, 0),
                )
                if h == 0 and warm_last is not None:
                    add_dep_helper(bm.ins, warm_last.ins, sync=True,
                                   reason="warmup precedes real stream")
                # w4 innermost: consecutive matmuls hit different PE column
                # groups so LDWEIGHTS overlaps the neighboring group's MATMUL
                for g in range(NG):
                    for j in range(KK):
                        for w4 in range(4):
                            wo = h * HF + g * 384 + w4 * 32
                            xo = (h * W2 + 4 * g + w4) * B
                            nc.tensor.matmul(
                                ps[32 * w4:32 * w4 + 32, 32 * g:32 * g + 32],
                                wt[:, wo + j * 128:wo + j * 128 + 32],
                                xt[:, xo + j * B:xo + j * B + 32],
                                start=False,
                                stop=(j == KK - 1),
                                skip_group_check=True,
                                tile_position=(0, 32 * w4),
                            )
                # stores in (4, 2, 2) row groups: short final store
                if h in (0, 4, 6):
                    onr = 4 if h == 0 else 2
                    obase = h * NG * 32
                    ot = opool.tile([128, onr * NG * 32], f16, tag="o")
                off = h * NG * 32 - obase
                if h == 7:
                    # tail off the busy scalar queue: copy on the idle
                    # vector engine, store from the idle sync ring
                    nc.vector.tensor_copy(ot[:, off:off + NG * 32], ps[:])
                    nc.sync.dma_start(
                        op.ap()[:, obase:(h + 1) * NG * 32], ot[:])
                else:
                    nc.scalar.copy(ot[:, off:off + NG * 32], ps[:])
                    if h in (3, 5):
                        nc.scalar.dma_start(
                            op.ap()[:, obase:(h + 1) * NG * 32], ot[:])
    nc.compile()
    _built["nc"] = nc
    return nc


def prep_inputs(x, weights, bias):
    """Host-side shard + layout prep. Returns list of 8 in_maps."""
    x = np.asarray(x, dtype=np.float32)
    weights = np.asarray(weights, dtype=np.float32)
    bias = np.asarray(bias, dtype=np.float32)
    xpad = np.zeros((B, C, H + 2, W2), dtype=np.float32)
    xpad[:, :, 1:H + 1, 1:W + 1] = x
    ind = np.kron(np.eye(16, dtype=np.float16), np.ones(32, dtype=np.float16))
    in_maps = []
    for d in range(NCORES):
        blk = xpad[:, :, RP * d:RP * d + RP + 2, :]       # [b, c, 10, 66]
        xprep = np.empty((C, KK, RP, W2, B), dtype=np.float16)
        for i in range(KK):
            xprep[:, i] = blk[:, :, i:i + RP, :].transpose(1, 2, 3, 0)
        xprep = xprep.reshape(P, RP * W2 * B)

        wd = weights[RP * d:RP * d + RP]                  # [8, 64, 32, 32, 3, 3]
        wd = wd.reshape(RP, NG, 4, O, C, KK, KK)          # h, g, w4, o, c, i, j
        wci = wd.transpose(4, 5, 0, 1, 6, 2, 3)           # c, i, h, g, j, w4, o
        import ml_dtypes
        wdt = np.float16 if W_FP16 else ml_dtypes.float8_e4m3
        wprep = np.ascontiguousarray(wci).astype(wdt).reshape(
            P, RP * NG * 384)

        bd = bias[:, RP * d:RP * d + RP, :].reshape(O, RP, NG, 4)
        U = bd.transpose(2, 1, 3, 0).reshape(NG, RP * 128)  # g, (h, w4, o)
        bprep = np.concatenate(
            [U.astype(np.float16), ind], axis=1)            # [16, 8*128+512]
        in_maps.append({"xp": xprep, "wp": wprep, "bp": bprep})
    return in_maps


def assemble_output(results):
    """results: list of 8 dicts with 'op' [128, 8*512] -> full [B,O,H,W]."""
    out = np.empty((B, O, H, W), dtype=np.float32)
    for d in range(NCORES):
        arr = np.asarray(results[d]["op"]).reshape(4, O, RP, NG, B)
        # [w4, o, h, g, b] -> [b, o, h, g, w4]
        out[:, :, RP * d:RP * d + RP, :] = (
            arr.transpose(4, 1, 2, 3, 0).reshape(B, O, RP, W)
            .astype(np.float32))
    return out


def _ensure_ntff_hook():
    """The agent image's antenv lacks axon_hooks; inject it and register the
    ctypes NTFF hook (same recipe as trn_agent_boot.trn_boot)."""
    try:
        from antenv.axon_hooks import get_axon_ntff_profile_hook  # noqa: F401
        return
    except ImportError:
        pass
    import types
    import ctypes
    import contextlib

    mod = types.ModuleType("antenv.axon_hooks")
    mod._hook = None

    def set_axon_ntff_profile_hook(h):
        mod._hook = h

    def get_axon_ntff_profile_hook():
        return mod._hook

    mod.set_axon_ntff_profile_hook = set_axon_ntff_profile_hook
    mod.get_axon_ntff_profile_hook = get_axon_ntff_profile_hook
    sys.modules["antenv.axon_hooks"] = mod
    import antenv

    antenv.axon_hooks = mod

    so_path = "/opt/axon/libaxon_pjrt.so"
    try:
        lib = ctypes.CDLL(so_path)
    except OSError:
        return
    if not hasattr(lib, "axon_start_nrt_profile"):
        return
    lib.axon_start_nrt_profile.argtypes = [
        ctypes.POINTER(ctypes.c_int64), ctypes.c_size_t]
    lib.axon_start_nrt_profile.restype = ctypes.c_int64
    lib.axon_stop_nrt_profile.argtypes = [ctypes.c_char_p]
    lib.axon_stop_nrt_profile.restype = ctypes.c_int64

    @contextlib.contextmanager
    def _hook(output_dir, device_ids):
        import jax

        jax.devices()
        if device_ids:
            ids = (ctypes.c_int64 * len(device_ids))(*device_ids)
            rc = lib.axon_start_nrt_profile(ids, len(device_ids))
        else:
            rc = lib.axon_start_nrt_profile(None, 0)
        if rc != 0:
            raise RuntimeError(f"axon_start_nrt_profile rc={rc}")
        try:
            yield
        finally:
            n = lib.axon_stop_nrt_profile(str(output_dir).encode())
            print(f"ntff profile: {n} file(s) written to {output_dir}")

    mod.set_axon_ntff_profile_hook(_hook)


def run(inputs, trace=False, **kwargs):
    from concourse.bass_utils import run_bass_kernel_spmd

    if trace:
        _ensure_ntff_hook()
    nc = _build()
    in_maps = prep_inputs(inputs["x"], inputs["weights"], inputs["bias"])
    res = run_bass_kernel_spmd(nc, in_maps, list(range(NCORES)),
                               trace=trace, **kwargs)
    return assemble_output(res.results), res


def kernel(**inputs):
    out, _ = run(inputs)
    return out


# revision 34
# speedup vs baseline: 1.1057x; 1.0605x over previous
"""LocallyConnected2d Trainium2 kernel.

Problem: out[b,o,h,w] = sum_{c,i,j} xpad[b,c,h+i,w+j] * weights[h,w,o,c,i,j] + bias[o,h,w]
  B=32, C=32, O=32, H=W=64, K=3, PAD=1, OH=OW=64.

Sharding: each of the 8 cores owns a band of 8 output rows (OH split), with the
matching 10-row input halo. Weights (the dominant traffic) split 1/8 per core
with zero redundancy, streamed as fp16.

Device compute: per output location (h,w) the contraction over (c,i,j)=288 is
split into 3 matmuls of K=96 (tap col j fixed, contraction over (c,i)),
accumulated in PSUM. M=o=32, N=b=32. The 4 locations of an ow-group share the
PE array via col-group tile_position packing. Bias is applied by one rank-16
matmul per output row: lhsT = bias[16g, 128(w4,o)], rhs = g-indicator
[16, 512(g,b)], writing the initial PSUM contents (start=True). The scalar
(activation) engine downcasts PSUM fp32 -> fp16 SBUF tiles, DMAed out in two
0.5 MiB stores. All layout transforms are host-side numpy; every DMA is a
clean 96/16/128-partition spray (one sem per sync-queue load, no recycling).

Host-prepped per-core layouts:
  xp [96, 8*66*32]  : [(c,i), r', w', b] = xpad[b, c, 8d+r'+i, w']
  wp [96, 8*16*384] : [(c,i), h, g, j, w4, o] = weights[8d+h, 4g+w4, o, c, i, j]
  bp [16, 8*128+512]: [g, (h, w4, o)] = bias[o, 8d+h, 4g+w4] ++ kron(I16, 1_32)
  op [2, 128, 2048] : [s, (w4,o), hh, g, b] fp16 (output, h = 4s+hh)
"""

import sys

if "/opt/trn_rl_repo" not in sys.path:
    sys.path.insert(0, "/opt/trn_rl_repo")

import numpy as np

B = 32
C = 32
O = 32
H = W = 64
KK = 3
NCORES = 8
RP = H // NCORES      # output rows per core
W2 = W + 2            # padded row width
P = 96                # contraction partitions (c,i)
NG = W // 4           # ow groups of 4
XGROUPS = [(0, 1), (1, 3), (4, 4)]          # x load groups (start row, nrows)
NWARM = 0                                   # PE warmup matmuls (DVFS pre-ramp)
BCOLS = RP * 128 + 512                      # bias tensor free dim
CHAIN = 2                                   # load chaining depth (0 = off)
W_FP16 = False
W_PAIRS = False                              # weights dtype: fp16 vs fp8e4

_built = {}


def _build():
    if "nc" in _built:
        return _built["nc"]
    import concourse.tile as tile
    from concourse.tile import add_dep_helper
    from concourse import bacc, mybir

    nc = bacc.Bacc("TRN2", target_bir_lowering=False, debug=False,
                   num_devices=NCORES)
    f16 = mybir.dt.float16
    f32 = mybir.dt.float32
    f8 = mybir.dt.float8e4
    WDT = f16 if W_FP16 else f8
    xp = nc.dram_tensor("xp", [P, RP * W2 * B], f16, kind="ExternalInput")
    wp = nc.dram_tensor("wp", [P, RP * NG * 384], WDT, kind="ExternalInput")
    bp = nc.dram_tensor("bp", [16, BCOLS], f16, kind="ExternalInput")
    op = nc.dram_tensor("op", [128, RP * NG * 32], f16,
                        kind="ExternalOutput")

    HF = NG * 384        # free elems per h row in wp
    with tile.TileContext(nc) as tc:
        with tc.tile_pool(name="xpool", bufs=1) as xpool, \
             tc.tile_pool(name="wpool", bufs=1) as wpool, \
             tc.tile_pool(name="bpool", bufs=1) as bpool, \
             tc.tile_pool(name="opool", bufs=3) as opool, \
             tc.tile_pool(name="ppool", bufs=4, space="PSUM") as ppool:
            xt = xpool.tile([P, RP * W2 * B], f16, tag="x")
            wt = wpool.tile([P, RP * HF], WDT, tag="w")
            bt = bpool.tile([16, BCOLS], f16, tag="b")
            wu = bpool.tile([1, 64], f16, tag="wu")

            # memzero first on the scalar queue: the PE warmup stream
            # depends only on it, so warmups start ~1us into the kernel.
            nc.scalar.memzero(wu[:])

            # loads: sync queue = exactly the 8 weight loads (8 HWDGE sems,
            # no slot recycling); scalar queue = bias + x + copies + stores.
            nc.scalar.dma_start(bt[:], bp.ap())

            loadq = []

            def load_w(h, n=1):
                cols = slice(h * HF, (h + n) * HF)
                loadq.append(nc.sync.dma_start(wt[:, cols], wp.ap()[:, cols]))

            def load_xg(gi):
                r0, nr = XGROUPS[gi]
                cols = slice(r0 * W2 * B, (r0 + nr) * W2 * B)
                loadq.append(nc.scalar.dma_start(xt[:, cols], xp.ap()[:, cols]))

            if W_PAIRS:
                load_w(0, 2)
                load_xg(0)
                load_w(2, 2)
                load_xg(1)
                load_w(4, 2)
                load_xg(2)
                load_w(6, 2)
            else:
                load_w(0)
                load_xg(0)
                load_w(1)
                load_xg(1)
                load_w(2)
                load_xg(2)
                for h in range(3, RP):
                    load_w(h)
            if CHAIN:
                for k in range(CHAIN, len(loadq)):
                    add_dep_helper(loadq[k].ins, loadq[k - CHAIN].ins,
                                   sync=True, reason="load chain")

            # PE DVFS warmup: dummy matmuls on zeroed scratch keep the
            # tensor engine busy while the first loads land, so the real
            # matmul stream starts at full clock.
            warm_last = None
            pw = None
            if NWARM:
                pw = ppool.tile([32, 32], f32, tag="ps")
            for k in range(NWARM):
                warm_last = nc.tensor.matmul(
                    pw[:, :], wu[0:1, 0:32], wu[0:1, 32:64],
                    start=True, stop=True, skip_group_check=True,
                    tile_position=(0, 0),
                )

            ot = None
            for h in range(RP):
                ps = ppool.tile([128, NG * 32], f32, tag="ps")
                bm = nc.tensor.matmul(
                    ps[:, :],
                    bt[:, h * 128:(h + 1) * 128],
                    bt[:, RP * 128:RP * 128 + 512],
                    start=True, stop=False,
                    skip_group_check=True,
                    tile_position=(0, 0),
                )
                if h == 0 and warm_last is not None:
                    add_dep_helper(bm.ins, warm_last.ins, sync=True,
                                   reason="warmup precedes real stream")
                # w4 innermost: consecutive matmuls hit different PE column
                # groups so LDWEIGHTS overlaps the neighboring group's MATMUL
                for g in range(NG):
                    for j in range(KK):
                        for w4 in range(4):
                            wo = h * HF + g * 384 + w4 * 32
                            xo = (h * W2 + 4 * g + w4) * B
                            nc.tensor.matmul(
                                ps[32 * w4:32 * w4 + 32, 32 * g:32 * g + 32],
                                wt[:, wo + j * 128:wo + j * 128 + 32],
                                xt[:, xo + j * B:xo + j * B + 32],
                                start=False,
                                stop=(j == KK - 1),
                                skip_group_check=True,
                                tile_position=(0, 32 * w4),
                            )
                # stores in (4, 2, 2) row groups: short final store
                if h in (0, 4, 6):
                    onr = 4 if h == 0 else 2
                    obase = h * NG * 32
                    ot = opool.tile([128, onr * NG * 32], f16, tag="o")
                off = h * NG * 32 - obase
                nc.scalar.copy(ot[:, off:off + NG * 32], ps[:])
                if h in (3, 5, 7):
                    nc.scalar.dma_start(
                        op.ap()[:, obase:(h + 1) * NG * 32], ot[:])
    nc.compile()
    _built["nc"] = nc
    return nc


def prep_inputs(x, weights, bias):
    """Host-side shard + layout prep. Returns list of 8 in_maps."""
    x = np.asarray(x, dtype=np.float32)
    weights = np.asarray(weights, dtype=np.float32)
    bias = np.asarray(bias, dtype=np.float32)
    xpad = np.zeros((B, C, H + 2, W2), dtype=np.float32)
    xpad[:, :, 1:H + 1, 1:W + 1] = x
    ind = np.kron(np.eye(16, dtype=np.float16), np.ones(32, dtype=np.float16))
    in_maps = []
    for d in range(NCORES):
        blk = xpad[:, :, RP * d:RP * d + RP + 2, :]       # [b, c, 10, 66]
        xprep = np.empty((C, KK, RP, W2, B), dtype=np.float16)
        for i in range(KK):
            xprep[:, i] = blk[:, :, i:i + RP, :].transpose(1, 2, 3, 0)
        xprep = xprep.reshape(P, RP * W2 * B)

        wd = weights[RP * d:RP * d + RP]                  # [8, 64, 32, 32, 3, 3]
        wd = wd.reshape(RP, NG, 4, O, C, KK, KK)          # h, g, w4, o, c, i, j
        wci = wd.transpose(4, 5, 0, 1, 6, 2, 3)           # c, i, h, g, j, w4, o
        import ml_dtypes
        wdt = np.float16 if W_FP16 else ml_dtypes.float8_e4m3
        wprep = np.ascontiguousarray(wci).astype(wdt).reshape(
            P, RP * NG * 384)

        bd = bias[:, RP * d:RP * d + RP, :].reshape(O, RP, NG, 4)
        U = bd.transpose(2, 1, 3, 0).reshape(NG, RP * 128)  # g, (h, w4, o)
        bprep = np.concatenate(
            [U.astype(np.float16), ind], axis=1)            # [16, 8*128+512]
        in_maps.append({"xp": xprep, "wp": wprep, "bp": bprep})
    return in_maps


def assemble_output(results):
    """results: list of 8 dicts with 'op' [128, 8*512] -> full [B,O,H,W]."""
    out = np.empty((B, O, H, W), dtype=np.float32)
    for d in range(NCORES):
        arr = np.asarray(results[d]["op"]).reshape(4, O, RP, NG, B)
        # [w4, o, h, g, b] -> [b, o, h, g, w4]
        out[:, :, RP * d:RP * d + RP, :] = (
            arr.transpose(4, 1, 2, 3, 0).reshape(B, O, RP, W)
            .astype(np.float32))
    return out


def _ensure_ntff_hook():
    """The agent image's antenv lacks axon_hooks; inject it and register the
    ctypes NTFF hook (same recipe as trn_agent_boot.trn_boot)."""
    try:
        from antenv.axon_hooks import get_axon_ntff_profile_hook  # noqa: F401
        return
    except ImportError:
        pass
    import types
    import ctypes
    import contextlib

    mod = types.ModuleType("antenv.axon_hooks")
    mod._hook = None

    def set_axon_ntff_profile_hook(h):
        mod._hook = h

    def get_axon_ntff_profile_hook():
        return mod._hook

    mod.set_axon_ntff_profile_hook = set_axon_ntff_profile_hook
    mod.get_axon_ntff_profile_hook = get_axon_ntff_profile_hook
    sys.modules["antenv.axon_hooks"] = mod
    import antenv

    antenv.axon_hooks = mod

    so_path = "/opt/axon/libaxon_pjrt.so"
    try:
        lib = ctypes.CDLL(so_path)
    except OSError:
        return
    if not hasattr(lib, "axon_start_nrt_profile"):
        return
    lib.axon_start_nrt_profile.argtypes = [
        ctypes.POINTER(ctypes.c_int64), ctypes.c_size_t]
    lib.axon_start_nrt_profile.restype = ctypes.c_int64
    lib.axon_stop_nrt_profile.argtypes = [ctypes.c_char_p]
    lib.axon_stop_nrt_profile.restype = ctypes.c_int64

    @contextlib.contextmanager
    def _hook(output_dir, device_ids):
        import jax

        jax.devices()
        if device_ids:
            ids = (ctypes.c_int64 * len(device_ids))(*device_ids)
            rc = lib.axon_start_nrt_profile(ids, len(device_ids))
        else:
            rc = lib.axon_start_nrt_profile(None, 0)
        if rc != 0:
            raise RuntimeError(f"axon_start_nrt_profile rc={rc}")
        try:
            yield
        finally:
            n = lib.axon_stop_nrt_profile(str(output_dir).encode())
            print(f"ntff profile: {n} file(s) written to {output_dir}")

    mod.set_axon_ntff_profile_hook(_hook)


def run(inputs, trace=False, **kwargs):
    from concourse.bass_utils import run_bass_kernel_spmd

    if trace:
        _ensure_ntff_hook()
    nc = _build()
    in_maps = prep_inputs(inputs["x"], inputs["weights"], inputs["bias"])
    res = run_bass_kernel_spmd(nc, in_maps, list(range(NCORES)),
                               trace=trace, **kwargs)
    return assemble_output(res.results), res


def kernel(**inputs):
    out, _ = run(inputs)
    return out
